# revision 71
# baseline (speedup 1.0000x reference)
"""Single attention head (B=8, S=2048, D=768, H=12) on 8 TRN2 NeuronCores.

Data-parallel over batch (1 element/core). Host prep is layout only:
  - per-batch permutation placing masked-in keys first (key extent compacts
    from 2048 to T_pad ~ 1152),
  - x transposed to (D, S), scaled by 16, split into fp16 hi/lo limbs, and
    stored chunk-blocked so each DMA line is one long contiguous run,
  - weights packed [Wk | Wq/sqrt(H) | Wv] at 32-aligned columns, scaled by
    64, split into fp16 limbs,
  - mask converted to a NEGATED additive bias row (0 / +60000).

Device pipeline per core (single score pass; ACT-paced):
  1. qkvT (96p, chunk) = 3 limb passes of W^T @ xT in PSUM, t-range chunks
     first so K/V are ready early.  Egress negated q rows (hi/lo) and
     positive k rows (hi/lo) + v as fp16.
  2. scores: per 128-row s-tile, ONE 37-row matmul streams all T_pad cols:
     rows (-qh|-ql|-qh)x(kh|kh|kl) + 1 x negbias -> PSUM holds -s.
  3. DVE reduce_min -> -m; ACT exp(-1*psum + (-m)) with per-partition bias
     -> p tile fp16 (softmax numerator, unnormalized).
  4. DMA-xbar transpose p -> pT (t-partition layout), off the PE/DVE/ACT.
  5. AV: 4-way column-tiled matmuls [v|1]^T @ pT accumulate per col-group;
     PSUM -> DRAM raw; host sums col groups, divides by the ones-row, and
     inverts the permutation.
"""

import math
import os

import numpy as np

B, S, D, H = 8, 2048, 768, 12
N_CORES = 8
BIAS_NEG = 60000.0       # negated additive mask bias
DS = 2.0 ** -10          # descale after limb matmuls (x*16, w*64)


def _ranges(lo, hi, step):
    out = []
    c = lo
    while c < hi:
        out.append((c, min(c + step, hi)))
        c = min(c + step, hi)
    return out


def _build(nc_mod, T_pad, T_act):
    bass, mybir, tile, bacc = nc_mod
    f32 = mybir.dt.float32
    f16 = mybir.dt.float16
    f8 = mybir.dt.float8e4
    AF = mybir.ActivationFunctionType
    OP = mybir.AluOpType
    X = mybir.AxisListType.X

    NT = T_pad // 128            # t tiles
    NS = S // 128                # s tiles (16)
    tchunks = _ranges(0, T_pad, 512)         # proj chunks covering key range
    schunks = _ranges(T_pad, S, 512)         # proj chunks, query-only range
    chunks = tchunks + schunks
    tsplits = _ranges(0, T_act, 512)         # per-s-tile score sub-matmuls

    nc = bacc.Bacc("TRN2", target_bir_lowering=False, debug=False,
                   num_devices=N_CORES)

    xh_ext = nc.dram_tensor("xh", [128, 6 * S], f16, kind="ExternalInput")
    xl_ext = nc.dram_tensor("xl", [128, 6 * S], f8, kind="ExternalInput")
    w_ext = nc.dram_tensor("w", [128, 6 * 192], f16, kind="ExternalInput")
    w8_ext = nc.dram_tensor("w8", [128, 6 * 96], f8, kind="ExternalInput")
    nbias_ext = nc.dram_tensor("nbias", [2, T_pad], f16, kind="ExternalInput")
    out_ext = nc.dram_tensor("out", [128, S], f32, kind="ExternalOutput")

    with tile.TileContext(nc) as tc:
        with tc.tile_pool(name="sb", bufs=1) as sb:

            xh = sb.tile([128, 6, S], f16)
            # lo limb, scaled 2^9, fp8, DoubleRow-interleaved k-tile pairs
            xl = sb.tile([128, 3, 2, S], f8)
            w = sb.tile([128, 6, 192], f16)    # [wh(96) | wl(96)] per k-tile
            w8 = sb.tile([128, 3, 2, 96], f8)  # wh * 2^-9, fp8, DR pairs
            # kTb rows: 0-11 kh, 12-23 kh dup, 24-35 kl, 36 negbias
            kTb = sb.tile([128, T_pad], f16)
            # rhsB rows: 0-11 -qh, 12-23 -ql, 24-35 -qh dup, 36 = +1
            rhsB = sb.tile([128, S], f16)
            qlst = sb.tile([12, S], f16)       # -ql staging (partition 0-11)
            klst = sb.tile([12, T_pad], f16)   # kl staging
            vsb = sb.tile([16, T_pad], f16)    # v rows + ones row 12
            vaug = sb.tile([128, NT, 16], f16)
            negm = sb.tile([128, NS], f32)     # -rowmax per s-tile
            osb = sb.tile([128, S], f32)       # AV psum egress staging
            pbufs = [sb.tile([128, T_pad], f16, name=f"pbuf{r}")
                     for r in range(3)]
            pT = sb.tile([128, NT, S], f16)    # transposed softmax numerators
            wrm = sb.tile([1, 1], f32)

            # ACT exp table preload at t~0 (one dummy exp)
            nc.gpsimd.memset(wrm[:], 0.0)
            nc.scalar.activation(wrm[:], wrm[:], AF.Exp)

            nc.gpsimd.memset(rhsB[:, :], 1.0)   # row 36 stays +1
            # kTb rows 37-127 stay 0.0: scores matmul runs K=128 (full row
            # activity keeps the HAM clock-gate warm) with zero contribution
            nc.gpsimd.memset(kTb[:, :], 0.0)
            nc.gpsimd.memset(vsb[:, :], 0.0)
            for r in range(3):
                # cols T_act..T_pad stay zero (masked in every batch)
                nc.gpsimd.memset(pbufs[r][:, :], 0.0)

            nc.sync.dma_start(w[:], w_ext.ap().rearrange(
                "p (ko m) -> p ko m", m=192))
            nc.sync.dma_start(w8[:], w8_ext.ap().rearrange(
                "p (a b m) -> p a b m", b=2, m=96))
            off = 0
            for ci, (c0, c1) in enumerate(chunks):
                lc = c1 - c0
                src = xh_ext.ap()[:, off:off + 6 * lc].rearrange(
                    "p (ko s) -> p ko s", s=lc)
                nc.sync.dma_start(xh[:, :, c0:c1], src)
                src = xl_ext.ap()[:, off:off + 6 * lc].rearrange(
                    "p (a b s) -> p a b s", b=2, s=lc)
                nc.sync.dma_start(xl[:, :, :, c0:c1], src)
                off += 6 * lc
                if ci == len(tchunks) - 1:
                    nc.sync.dma_start(kTb[36:37, :], nbias_ext.ap()[0:1, :])
                    nc.sync.dma_start(vsb[12:13, :], nbias_ext.ap()[1:2, :])

            def proj_pass(ps, c0, c1, pas):
                lc = c1 - c0
                if pas == 2:
                    for sko in range(3):   # fp8 DoubleRow: 2 k-tiles/matmul
                        nc.tensor.matmul(
                            ps[0:96, 0:lc], w8[:, sko, :, :],
                            xl[:, sko, :, c0:c1],
                            start=False, stop=(sko == 2),
                            perf_mode=mybir.MatmulPerfMode.DoubleRow)
                    return
                for ko in range(6):
                    wsl = slice(96, 192) if pas == 1 else slice(0, 96)
                    nc.tensor.matmul(
                        ps[0:96, 0:lc], w[:, ko, wsl], xh[:, ko, c0:c1],
                        start=(pas == 0 and ko == 0), stop=False)

            def q_egress(ps, c0, c1):
                lc = c1 - c0
                # hi-limb rounding on the scalar engine, residual on DVE:
                # halves the serial egress latency per chunk
                nc.scalar.mul(rhsB[0:12, c0:c1], ps[32:44, 0:lc], -DS)
                nc.vector.scalar_tensor_tensor(
                    qlst[:, c0:c1], ps[32:44, 0:lc], -DS,
                    rhsB[0:12, c0:c1], op0=OP.mult, op1=OP.subtract)

            def kv_egress(ps, c0, c1):
                lc = c1 - c0
                nc.scalar.mul(kTb[0:12, c0:c1], ps[0:12, 0:lc], DS)
                nc.vector.scalar_tensor_tensor(
                    klst[:, c0:c1], ps[0:12, 0:lc], DS,
                    kTb[0:12, c0:c1], op0=OP.mult, op1=OP.subtract)
                nc.scalar.mul(vsb[0:12, c0:c1], ps[64:76, 0:lc], DS)

            def emit_score(i, scp):
                s0 = i * 128
                sp = scp.tile([128, 1536], f32, tag="sc")
                for (t0, t1) in tsplits:
                    nc.tensor.matmul(
                        sp[:, t0:t1], rhsB[0:128, s0:s0 + 128],
                        kTb[0:128, t0:t1], start=True, stop=True)
                nc.vector.tensor_reduce(
                    negm[:, i:i + 1], sp[:, 0:T_act], axis=X, op=OP.min)
                p = pbufs[i % 3]
                # two ACT calls: halves the psum WAR granularity, so the
                # next-next tile's first score matmul can start sooner
                nc.scalar.activation(
                    p[:, 0:512], sp[:, 0:512], AF.Exp,
                    bias=negm[:, i:i + 1], scale=-1.0)
                nc.scalar.activation(
                    p[:, 512:T_act], sp[:, 512:T_act], AF.Exp,
                    bias=negm[:, i:i + 1], scale=-1.0)
                nc.sync.dma_start_transpose(pT[:, :, s0:s0 + 128], p[:])

            av_state = {}

            def emit_av(k, avp):
                c = k // 4
                if k % 4 == 0:
                    av_state[c] = avp.tile([128, 512], f32, tag="av",
                                           name=f"vac_{c}")
                vac = av_state[c]
                col = (k % 4) * 128
                for j in range(NT):
                    g = j % 4
                    nc.tensor.matmul(
                        vac[32 * g:32 * g + 13, col:col + 128],
                        vaug[:, j, 0:13], pT[:, j, k * 128:(k + 1) * 128],
                        start=(j < 4), stop=(j + 4 >= NT),
                        tile_position=(0, 32 * g))
                if k % 4 == 3:
                    nc.vector.tensor_copy(
                        osb[:, c * 512:(c + 1) * 512], vac[:])

            with tc.tile_pool(name="scp", bufs=2, space="PSUM") as scp:
                with tc.tile_pool(name="projp", bufs=2, space="PSUM") as projp:
                    # dummy matmuls fill the input-DMA wait: HAM un-throttles
                    # the PE to 2.4 GHz before the real projection starts
                    # (ping-pong two psum banks so they stream, not WAW-stall)
                    ps_w = [projp.tile([96, 512], f32, tag="ps",
                                       name=f"warm{r}") for r in range(2)]
                    w2d = w[:].rearrange("p a b -> p (a b)")
                    for t in range(11):
                        nc.tensor.matmul(
                            ps_w[t % 2][0:96, :], w2d[:, 0:96],
                            w2d[:, 512:1024], start=True, stop=True)
                    for ci, (c0, c1) in enumerate(tchunks):
                        ps = projp.tile([96, 512], f32, tag="ps")
                        for pas in range(3):
                            proj_pass(ps, c0, c1, pas)
                        q_egress(ps, c0, c1)
                        kv_egress(ps, c0, c1)
                        # per-chunk row dup / shift copies on idle queues
                        cs = slice(c0, c1)
                        nc.scalar.dma_start(kTb[12:24, cs], kTb[0:12, cs])
                        nc.scalar.dma_start(kTb[24:36, cs], klst[:, cs])
                        nc.sync.dma_start(rhsB[12:24, cs], qlst[:, cs])
                        nc.sync.dma_start(rhsB[24:36, cs], rhsB[0:12, cs])
                        if ci == len(tchunks) - 1:
                            nc.scalar.dma_start_transpose(vaug[:], vsb[:])

                    # s-range proj in the same pool (no handoff stall).
                    # The LAST fp8 pass (waits the late xl tail DMA) is
                    # deferred past the scores-0-7 emission so its wait
                    # doesn't block the score pipeline in the PE queue.
                    deferred = []
                    for ci, (c0, c1) in enumerate(schunks):
                        ps = projp.tile([96, 512], f32, tag="ps",
                                        name=f"ps2_{ci}")
                        last = ci == len(schunks) - 1
                        for pas in range(2 if last else 3):
                            proj_pass(ps, c0, c1, pas)

                        def fin(ps=ps, c0=c0, c1=c1):
                            q_egress(ps, c0, c1)
                            cs = slice(c0, c1)
                            # software DGE: keeps the sync queue free for
                            # the latency-critical p-transposes
                            nc.gpsimd.dma_start(
                                rhsB[12:24, cs], qlst[:, cs])
                            nc.gpsimd.dma_start(
                                rhsB[24:36, cs], rhsB[0:12, cs])
                        if last:
                            deferred.append((ps, c0, c1, fin))
                        else:
                            fin()

                    for i in range(8):
                        emit_score(i, scp)
                    for (ps, c0, c1, fin) in deferred:
                        proj_pass(ps, c0, c1, 2)
                        fin()

                with tc.tile_pool(name="avp", bufs=1, space="PSUM") as avp:
                    for i in range(8, NS):
                        emit_score(i, scp)
                        emit_av(2 * (i - 8), avp)
                        emit_av(2 * (i - 8) + 1, avp)
                    nc.sync.dma_start(out_ext.ap(), osb[:])

    nc.compile()
    return nc


def kernel(x, mask, key_weight, query_weight, value_weight):
    import concourse.bass as bass
    import concourse.mybir as mybir
    import concourse.tile as tile
    from concourse import bacc, bass_utils

    x = np.asarray(x, dtype=np.float32)
    mask = np.asarray(mask)
    wk = np.asarray(key_weight, dtype=np.float32)
    wq = np.asarray(query_weight, dtype=np.float32)
    wv = np.asarray(value_weight, dtype=np.float32)

    # natural-units W, 32-aligned columns, x64 scale for fp16 limb split
    w2 = np.zeros((D, 96), dtype=np.float32)
    w2[:, 0:12] = wk
    w2[:, 32:44] = wq / math.sqrt(H)
    w2[:, 64:76] = wv
    import ml_dtypes
    f8np = ml_dtypes.float8_e4m3
    w2 *= 64.0
    wh = w2.astype(np.float16)
    wl = (w2 - wh.astype(np.float32)).astype(np.float16)
    w_cat = np.concatenate([wh, wl], axis=1)          # (768, 192) fp16
    w_host = np.ascontiguousarray(
        w_cat.reshape(6, 128, 192).transpose(1, 0, 2).reshape(128, 6 * 192))
    w8_host = np.ascontiguousarray(
        (w2 * 2.0 ** -9).astype(f8np).reshape(3, 2, 128, 96)
        .transpose(2, 0, 1, 3).reshape(128, 6 * 96))

    perms, nbs = [], []
    for b in range(B):
        m = mask[b, 0].astype(np.int64)
        perm = np.argsort(1 - m, kind="stable")
        perms.append(perm)
        nbs.append(int(m.sum()))
    T_pad = max(128, int(np.ceil(max(max(nbs), 1) / 128.0)) * 128)
    T_pad = min(T_pad, S)
    T_act = min(T_pad, (max(max(nbs), 1) + 7) // 8 * 8)

    chunks = _ranges(0, T_pad, 512) + _ranges(T_pad, S, 512)

    in_maps = []
    for b in range(B):
        xs = np.ascontiguousarray(x[b].T[:, perms[b]]) * 16.0
        xsh = xs.astype(np.float16)
        xsl = ((xs - xsh.astype(np.float32)) * 2.0 ** 9).astype(f8np)

        def blocked(a):   # (768, S) -> chunk-blocked (128, 6*S)
            a = a.reshape(6, 128, S).transpose(1, 0, 2)   # (128, 6, S)
            return np.concatenate(
                [np.ascontiguousarray(a[:, :, c0:c1]).reshape(128, -1)
                 for (c0, c1) in chunks], axis=1)

        def blocked_dr(a):  # fp8 DoubleRow pair-interleaved layout
            a = a.reshape(3, 2, 128, S).transpose(2, 0, 1, 3)  # (128,3,2,S)
            return np.concatenate(
                [np.ascontiguousarray(a[:, :, :, c0:c1]).reshape(128, -1)
                 for (c0, c1) in chunks], axis=1)

        nb_row = np.zeros((2, T_pad), dtype=np.float16)
        nb_row[0, nbs[b]:] = BIAS_NEG
        nb_row[1, :] = 1.0
        in_maps.append({"xh": blocked(xsh), "xl": blocked_dr(xsl),
                        "w": w_host, "w8": w8_host, "nbias": nb_row})

    import time as _time
    _t0 = _time.time()
    print(f"[kernel] building graph, T_pad={T_pad}", flush=True)
    nc = _build((bass, mybir, tile, bacc), T_pad, T_act)
    print(f"[kernel] graph+bacc compile done in {_time.time() - _t0:.1f}s",
          flush=True)

    trace = os.environ.get("BASS_KERNEL_TRACE", "0") == "1"
    if trace:
        import sys
        import types
        from trn_agent_boot.trn_boot import _ntff_profile_via_ctypes
        hook = _ntff_profile_via_ctypes("/opt/axon/libaxon_pjrt.so")
        m = types.ModuleType("antenv.axon_hooks")
        m.get_axon_ntff_profile_hook = lambda: hook
        sys.modules["antenv.axon_hooks"] = m
        bass_utils.upload_artifacts = lambda tmpdir: "local://" + tmpdir

    res = bass_utils.run_bass_kernel_spmd(
        nc, in_maps, core_ids=list(range(N_CORES)), trace=trace)
    if trace:
        print(f"HW exec time: {res.exec_time_ns} ns", flush=True)

    out = np.empty((B, S, H), dtype=np.float32)
    for b in range(B):
        r = res.results[b]["out"]                     # (128, S) f32
        aug = (r[0:13] + r[32:45] + r[64:77] + r[96:109]).astype(np.float64)
        o = (aug[0:12] / aug[12][None, :]).T          # (S, H)
        out[b, perms[b], :] = o.astype(np.float32)
    return out


# revision 72
# speedup vs baseline: 1.0835x; 1.0835x over previous
"""Single attention head (B=8, S=2048, D=768, H=12) on 8 TRN2 NeuronCores.

Data-parallel over batch (1 element/core). Host prep is layout only:
  - per-batch permutation placing masked-in keys first (key extent compacts
    from 2048 to T_pad ~ 1152),
  - x transposed to (D, S), scaled by 16, split into fp16 hi/lo limbs, and
    stored chunk-blocked so each DMA line is one long contiguous run,
  - weights packed [Wk | Wq/sqrt(H) | Wv] at 32-aligned columns, scaled by
    64, split into fp16 limbs,
  - mask converted to a NEGATED additive bias row (0 / +60000).

Device pipeline per core (single score pass; ACT-paced):
  1. qkvT (96p, chunk) = 3 limb passes of W^T @ xT in PSUM, t-range chunks
     first so K/V are ready early.  Egress negated q rows (hi/lo) and
     positive k rows (hi/lo) + v as fp16.
  2. scores: per 128-row s-tile, ONE 37-row matmul streams all T_pad cols:
     rows (-qh|-ql|-qh)x(kh|kh|kl) + 1 x negbias -> PSUM holds -s.
  3. DVE reduce_min -> -m; ACT exp(-1*psum + (-m)) with per-partition bias
     -> p tile fp16 (softmax numerator, unnormalized).
  4. DMA-xbar transpose p -> pT (t-partition layout), off the PE/DVE/ACT.
  5. AV: 4-way column-tiled matmuls [v|1]^T @ pT accumulate per col-group;
     PSUM -> DRAM raw; host sums col groups, divides by the ones-row, and
     inverts the permutation.
"""

import math
import os

import numpy as np

B, S, D, H = 8, 2048, 768, 12
N_CORES = 8
BIAS_NEG = 60000.0       # negated additive mask bias
DS = 2.0 ** -10          # descale after limb matmuls (x*16, w*64)


def _ranges(lo, hi, step):
    out = []
    c = lo
    while c < hi:
        out.append((c, min(c + step, hi)))
        c = min(c + step, hi)
    return out


def _build(nc_mod, T_pad, T_act):
    bass, mybir, tile, bacc = nc_mod
    f32 = mybir.dt.float32
    f16 = mybir.dt.float16
    f8 = mybir.dt.float8e4
    AF = mybir.ActivationFunctionType
    OP = mybir.AluOpType
    X = mybir.AxisListType.X

    NT = T_pad // 128            # t tiles
    NS = S // 128                # s tiles (16)
    tchunks = _ranges(0, T_pad, 512)         # proj chunks covering key range
    schunks = _ranges(T_pad, S, 512)         # proj chunks, query-only range
    chunks = tchunks + schunks
    tsplits = _ranges(0, T_act, 512)         # per-s-tile score sub-matmuls

    nc = bacc.Bacc("TRN2", target_bir_lowering=False, debug=False,
                   num_devices=N_CORES)

    xh_ext = nc.dram_tensor("xh", [128, 6 * S], f16, kind="ExternalInput")
    xl_ext = nc.dram_tensor("xl", [128, 6 * S], f8, kind="ExternalInput")
    w_ext = nc.dram_tensor("w", [128, 6 * 192], f16, kind="ExternalInput")
    w8_ext = nc.dram_tensor("w8", [128, 6 * 96], f8, kind="ExternalInput")
    nbias_ext = nc.dram_tensor("nbias", [2, T_pad], f16, kind="ExternalInput")
    out_ext = nc.dram_tensor("out", [128, S], f32, kind="ExternalOutput")

    with tile.TileContext(nc) as tc:
        with tc.tile_pool(name="sb", bufs=1) as sb:

            xh = sb.tile([128, 6, S], f16)
            # lo limb, scaled 2^9, fp8, DoubleRow-interleaved k-tile pairs
            xl = sb.tile([128, 3, 2, S], f8)
            w = sb.tile([128, 6, 192], f16)    # [wh(96) | wl(96)] per k-tile
            w8 = sb.tile([128, 3, 2, 96], f8)  # wh * 2^-9, fp8, DR pairs
            # kTb rows: 0-11 kh, 12-23 kh dup, 24-35 kl, 36 negbias
            kTb = sb.tile([128, T_pad], f16)
            # rhsB rows: 0-11 -qh, 12-23 -ql, 24-35 -qh dup, 36 = +1
            rhsB = sb.tile([128, S], f16)
            qlst = sb.tile([12, S], f16)       # -ql staging (partition 0-11)
            klst = sb.tile([12, T_pad], f16)   # kl staging
            vsb = sb.tile([16, T_pad], f16)    # v rows + ones row 12
            vaug = sb.tile([128, NT, 16], f16)
            negm = sb.tile([128, NS], f32)     # -rowmax per s-tile
            osb = sb.tile([128, S], f32)       # AV psum egress staging
            pbufs = [sb.tile([128, T_pad], f16, name=f"pbuf{r}")
                     for r in range(3)]
            pT = sb.tile([128, NT, S], f16)    # transposed softmax numerators
            wrm = sb.tile([1, 1], f32)

            # ACT exp table preload at t~0 (one dummy exp)
            nc.gpsimd.memset(wrm[:], 0.0)
            nc.scalar.activation(wrm[:], wrm[:], AF.Exp)

            nc.gpsimd.memset(rhsB[:, :], 1.0)   # row 36 stays +1
            # kTb rows 37-127 stay 0.0: scores matmul runs K=128 (full row
            # activity keeps the HAM clock-gate warm) with zero contribution
            nc.gpsimd.memset(kTb[:, :], 0.0)
            nc.gpsimd.memset(vsb[:, :], 0.0)
            for r in range(3):
                # cols T_act..T_pad stay zero (masked in every batch)
                nc.gpsimd.memset(pbufs[r][:, :], 0.0)

            nc.sync.dma_start(w[:], w_ext.ap().rearrange(
                "p (ko m) -> p ko m", m=192))
            nc.sync.dma_start(w8[:], w8_ext.ap().rearrange(
                "p (a b m) -> p a b m", b=2, m=96))
            off = 0
            for ci, (c0, c1) in enumerate(chunks):
                lc = c1 - c0
                src = xh_ext.ap()[:, off:off + 6 * lc].rearrange(
                    "p (ko s) -> p ko s", s=lc)
                nc.sync.dma_start(xh[:, :, c0:c1], src)
                src = xl_ext.ap()[:, off:off + 6 * lc].rearrange(
                    "p (a b s) -> p a b s", b=2, s=lc)
                nc.sync.dma_start(xl[:, :, :, c0:c1], src)
                off += 6 * lc
                if ci == len(tchunks) - 1:
                    nc.sync.dma_start(kTb[36:37, :], nbias_ext.ap()[0:1, :])
                    nc.sync.dma_start(vsb[12:13, :], nbias_ext.ap()[1:2, :])

            def proj_pass(ps, c0, c1, pas):
                lc = c1 - c0
                if pas == 2:
                    for sko in range(3):   # fp8 DoubleRow: 2 k-tiles/matmul
                        nc.tensor.matmul(
                            ps[0:96, 0:lc], w8[:, sko, :, :],
                            xl[:, sko, :, c0:c1],
                            start=False, stop=(sko == 2),
                            perf_mode=mybir.MatmulPerfMode.DoubleRow)
                    return
                for ko in range(6):
                    wsl = slice(96, 192) if pas == 1 else slice(0, 96)
                    nc.tensor.matmul(
                        ps[0:96, 0:lc], w[:, ko, wsl], xh[:, ko, c0:c1],
                        start=(pas == 0 and ko == 0), stop=False)

            def q_egress(ps, c0, c1):
                lc = c1 - c0
                nc.vector.tensor_scalar_mul(
                    rhsB[0:12, c0:c1], ps[32:44, 0:lc], -DS)
                nc.vector.scalar_tensor_tensor(
                    qlst[:, c0:c1], ps[32:44, 0:lc], -DS,
                    rhsB[0:12, c0:c1], op0=OP.mult, op1=OP.subtract)

            def kv_egress(ps, c0, c1):
                lc = c1 - c0
                nc.vector.tensor_scalar_mul(
                    kTb[0:12, c0:c1], ps[0:12, 0:lc], DS)
                nc.vector.scalar_tensor_tensor(
                    klst[:, c0:c1], ps[0:12, 0:lc], DS,
                    kTb[0:12, c0:c1], op0=OP.mult, op1=OP.subtract)
                nc.vector.tensor_scalar_mul(
                    vsb[0:12, c0:c1], ps[64:76, 0:lc], DS)

            def emit_score(i, scp):
                s0 = i * 128
                sp = scp.tile([128, 1536], f32, tag="sc")
                for (t0, t1) in tsplits:
                    nc.tensor.matmul(
                        sp[:, t0:t1], rhsB[0:128, s0:s0 + 128],
                        kTb[0:128, t0:t1], start=True, stop=True)
                nc.vector.tensor_reduce(
                    negm[:, i:i + 1], sp[:, 0:T_act], axis=X, op=OP.min)
                p = pbufs[i % 3]
                # two ACT calls: halves the psum WAR granularity, so the
                # next-next tile's first score matmul can start sooner
                nc.scalar.activation(
                    p[:, 0:512], sp[:, 0:512], AF.Exp,
                    bias=negm[:, i:i + 1], scale=-1.0)
                nc.scalar.activation(
                    p[:, 512:T_act], sp[:, 512:T_act], AF.Exp,
                    bias=negm[:, i:i + 1], scale=-1.0)
                nc.sync.dma_start_transpose(pT[:, :, s0:s0 + 128], p[:])

            av_state = {}

            def emit_av(k, avp):
                c = k // 4
                if k % 4 == 0:
                    av_state[c] = avp.tile([128, 512], f32, tag="av",
                                           name=f"vac_{c}")
                vac = av_state[c]
                col = (k % 4) * 128
                for j in range(NT):
                    g = j % 4
                    nc.tensor.matmul(
                        vac[32 * g:32 * g + 13, col:col + 128],
                        vaug[:, j, 0:13], pT[:, j, k * 128:(k + 1) * 128],
                        start=(j < 4), stop=(j + 4 >= NT),
                        tile_position=(0, 32 * g))
                if k % 4 == 3:
                    nc.vector.tensor_copy(
                        osb[:, c * 512:(c + 1) * 512], vac[:])

            with tc.tile_pool(name="scp", bufs=2, space="PSUM") as scp:
                with tc.tile_pool(name="projp", bufs=2, space="PSUM") as projp:
                    # dummy matmuls fill the input-DMA wait: HAM un-throttles
                    # the PE to 2.4 GHz before the real projection starts
                    # (ping-pong two psum banks so they stream, not WAW-stall)
                    ps_w = [projp.tile([96, 512], f32, tag="ps",
                                       name=f"warm{r}") for r in range(2)]
                    w2d = w[:].rearrange("p a b -> p (a b)")
                    for t in range(11):
                        nc.tensor.matmul(
                            ps_w[t % 2][0:96, :], w2d[:, 0:96],
                            w2d[:, 512:1024], start=True, stop=True)
                    for ci, (c0, c1) in enumerate(tchunks):
                        ps = projp.tile([96, 512], f32, tag="ps")
                        for pas in range(3):
                            proj_pass(ps, c0, c1, pas)
                        q_egress(ps, c0, c1)
                        kv_egress(ps, c0, c1)
                        # per-chunk row dup / shift copies on idle queues
                        cs = slice(c0, c1)
                        nc.scalar.dma_start(kTb[12:24, cs], kTb[0:12, cs])
                        nc.scalar.dma_start(kTb[24:36, cs], klst[:, cs])
                        nc.sync.dma_start(rhsB[12:24, cs], qlst[:, cs])
                        nc.sync.dma_start(rhsB[24:36, cs], rhsB[0:12, cs])
                        if ci == len(tchunks) - 1:
                            nc.scalar.dma_start_transpose(vaug[:], vsb[:])

                    # s-range proj in the same pool (no handoff stall).
                    # The LAST fp8 pass (waits the late xl tail DMA) is
                    # deferred past the scores-0-7 emission so its wait
                    # doesn't block the score pipeline in the PE queue.
                    deferred = []
                    for ci, (c0, c1) in enumerate(schunks):
                        ps = projp.tile([96, 512], f32, tag="ps",
                                        name=f"ps2_{ci}")
                        last = ci == len(schunks) - 1
                        for pas in range(2 if last else 3):
                            proj_pass(ps, c0, c1, pas)

                        def fin(ps=ps, c0=c0, c1=c1):
                            q_egress(ps, c0, c1)
                            cs = slice(c0, c1)
                            # software DGE: keeps the sync queue free for
                            # the latency-critical p-transposes
                            nc.gpsimd.dma_start(
                                rhsB[12:24, cs], qlst[:, cs])
                            nc.gpsimd.dma_start(
                                rhsB[24:36, cs], rhsB[0:12, cs])
                        if last:
                            deferred.append((ps, c0, c1, fin))
                        else:
                            fin()

                    for i in range(8):
                        emit_score(i, scp)
                    for (ps, c0, c1, fin) in deferred:
                        proj_pass(ps, c0, c1, 2)
                        fin()

                with tc.tile_pool(name="avp", bufs=1, space="PSUM") as avp:
                    for i in range(8, NS):
                        emit_score(i, scp)
                        emit_av(2 * (i - 8), avp)
                        emit_av(2 * (i - 8) + 1, avp)
                    nc.sync.dma_start(out_ext.ap(), osb[:])

    nc.compile()
    return nc


def kernel(x, mask, key_weight, query_weight, value_weight):
    import concourse.bass as bass
    import concourse.mybir as mybir
    import concourse.tile as tile
    from concourse import bacc, bass_utils

    x = np.asarray(x, dtype=np.float32)
    mask = np.asarray(mask)
    wk = np.asarray(key_weight, dtype=np.float32)
    wq = np.asarray(query_weight, dtype=np.float32)
    wv = np.asarray(value_weight, dtype=np.float32)

    # natural-units W, 32-aligned columns, x64 scale for fp16 limb split
    w2 = np.zeros((D, 96), dtype=np.float32)
    w2[:, 0:12] = wk
    w2[:, 32:44] = wq / math.sqrt(H)
    w2[:, 64:76] = wv
    import ml_dtypes
    f8np = ml_dtypes.float8_e4m3
    w2 *= 64.0
    wh = w2.astype(np.float16)
    wl = (w2 - wh.astype(np.float32)).astype(np.float16)
    w_cat = np.concatenate([wh, wl], axis=1)          # (768, 192) fp16
    w_host = np.ascontiguousarray(
        w_cat.reshape(6, 128, 192).transpose(1, 0, 2).reshape(128, 6 * 192))
    w8_host = np.ascontiguousarray(
        (w2 * 2.0 ** -9).astype(f8np).reshape(3, 2, 128, 96)
        .transpose(2, 0, 1, 3).reshape(128, 6 * 96))

    perms, nbs = [], []
    for b in range(B):
        m = mask[b, 0].astype(np.int64)
        perm = np.argsort(1 - m, kind="stable")
        perms.append(perm)
        nbs.append(int(m.sum()))
    T_pad = max(128, int(np.ceil(max(max(nbs), 1) / 128.0)) * 128)
    T_pad = min(T_pad, S)
    T_act = min(T_pad, (max(max(nbs), 1) + 7) // 8 * 8)

    chunks = _ranges(0, T_pad, 512) + _ranges(T_pad, S, 512)

    in_maps = []
    for b in range(B):
        xs = np.ascontiguousarray(x[b].T[:, perms[b]]) * 16.0
        xsh = xs.astype(np.float16)
        xsl = ((xs - xsh.astype(np.float32)) * 2.0 ** 9).astype(f8np)

        def blocked(a):   # (768, S) -> chunk-blocked (128, 6*S)
            a = a.reshape(6, 128, S).transpose(1, 0, 2)   # (128, 6, S)
            return np.concatenate(
                [np.ascontiguousarray(a[:, :, c0:c1]).reshape(128, -1)
                 for (c0, c1) in chunks], axis=1)

        def blocked_dr(a):  # fp8 DoubleRow pair-interleaved layout
            a = a.reshape(3, 2, 128, S).transpose(2, 0, 1, 3)  # (128,3,2,S)
            return np.concatenate(
                [np.ascontiguousarray(a[:, :, :, c0:c1]).reshape(128, -1)
                 for (c0, c1) in chunks], axis=1)

        nb_row = np.zeros((2, T_pad), dtype=np.float16)
        nb_row[0, nbs[b]:] = BIAS_NEG
        nb_row[1, :] = 1.0
        in_maps.append({"xh": blocked(xsh), "xl": blocked_dr(xsl),
                        "w": w_host, "w8": w8_host, "nbias": nb_row})

    import time as _time
    _t0 = _time.time()
    print(f"[kernel] building graph, T_pad={T_pad}", flush=True)
    nc = _build((bass, mybir, tile, bacc), T_pad, T_act)
    print(f"[kernel] graph+bacc compile done in {_time.time() - _t0:.1f}s",
          flush=True)

    trace = os.environ.get("BASS_KERNEL_TRACE", "0") == "1"
    if trace:
        import sys
        import types
        from trn_agent_boot.trn_boot import _ntff_profile_via_ctypes
        hook = _ntff_profile_via_ctypes("/opt/axon/libaxon_pjrt.so")
        m = types.ModuleType("antenv.axon_hooks")
        m.get_axon_ntff_profile_hook = lambda: hook
        sys.modules["antenv.axon_hooks"] = m
        bass_utils.upload_artifacts = lambda tmpdir: "local://" + tmpdir

    res = bass_utils.run_bass_kernel_spmd(
        nc, in_maps, core_ids=list(range(N_CORES)), trace=trace)
    if trace:
        print(f"HW exec time: {res.exec_time_ns} ns", flush=True)

    out = np.empty((B, S, H), dtype=np.float32)
    for b in range(B):
        r = res.results[b]["out"]                     # (128, S) f32
        aug = (r[0:13] + r[32:45] + r[64:77] + r[96:109]).astype(np.float64)
        o = (aug[0:12] / aug[12][None, :]).T          # (S, H)
        out[b, perms[b], :] = o.astype(np.float32)
    return out
